# revision 40
# baseline (speedup 1.0000x reference)
"""GRU-style GNN message-passing kernel for Trainium2 (8 NeuronCores, SPMD).

Reference computation (per node b, features 256, 8 neighbors):
    xr = x @ Wir.T + bir
    hr_n = hs_n @ Whr.T + bhr
    r_n = sigmoid(xr + hr_n)
    z = sigmoid(x @ Wiz.T + biz + h_sum @ Whz.T + bhz)
    s = sum_n r_n * hs_n
    n = tanh(x @ Win.T + bin + s @ Whn.T + bhn)
    out = (1 - z) * n + z * h_sum

Strategy: data-parallel over the node dim B=32768 across 8 cores (4096
rows each), batch-chunked 8x512 per core, feature-major on chip
(256 feats = 2 partition chunks of 128, batch in the free dim).

Performance structure:
  - All HBM tensors are host-packed into the exact SBUF image per chunk;
    the 13 weight/const tensors ride in 2 packed DMAs so the first
    chunk's data isn't stuck behind a serialized descriptor queue
    (HWDGE issue is ~650ns per dma_start). hs is split into two per-f
    DMAs so the first r-matmuls start before the full 2MB lands.
  - Software-pipelined emission: chunk c's n-gate + combine + store are
    emitted *between* the f=0 and f=1 r-unit groups of chunk c+1, so
    the PE stream never waits on the DVE product/sum tree.
  - Product+tree on DVE is split (neighbors 0-3 / 4-7) so it pipelines
    against the second half of the r-matmuls; the final combine runs on
    the otherwise-idle GPSIMD except for the last chunk (short tail).
  - PSUM: one pool of 4 x [128,1024] tiles (8 banks); xr, z, 8 r-units
    and the (delayed) n-gate rotate through it.
  - Dependency-free warm-up matmuls + activations at t=0 keep the PE
    HAM clock-gate and ACT tables warm through the initial DMA wait.
"""

import sys
import numpy as np
from contextlib import ExitStack

sys.path.insert(0, "/opt/trn_rl_repo")

import ml_dtypes
import concourse.bacc as bacc
import concourse.tile as tile
from concourse import mybir
from concourse.bass_utils import run_bass_kernel_spmd

F32 = mybir.dt.float32
BF16 = mybir.dt.bfloat16
BF_NP = ml_dtypes.bfloat16

N_NEIGH, B, IN, H = 8, 32768, 256, 256
M = 8                    # cores
BL = B // M              # rows per core (4096)
NCH = 8                  # batch chunks per core
CW = BL // NCH           # chunk width (512)
HSW = 2 * N_NEIGH * CW   # hs tile width (8192)

SIG = mybir.ActivationFunctionType.Sigmoid
TANH = mybir.ActivationFunctionType.Tanh
IDENT = mybir.ActivationFunctionType.Identity

# packed const layout: [id | wir0 wir1 wiz0 wiz1 whz0 whz1] / [whr0 whr1 win0 win1 whn0 whn1]
WA_ORDER = ("wir", "wiz", "whz")
WB_ORDER = ("whr", "win", "whn")
WA_COLS = 128 + 6 * 256
WB_COLS = 6 * 256

_cached = None  # compiled program, reused across kernel() calls


def _build():
    nc = bacc.Bacc("TRN2", target_bir_lowering=False, debug=False, num_devices=M)

    xD = nc.dram_tensor("xT", [NCH, 128, 2 * CW], BF16, kind="ExternalInput").ap()
    hD = nc.dram_tensor("hT", [NCH, 128, 2 * CW], F32, kind="ExternalInput").ap()
    hbD = nc.dram_tensor("hbT", [NCH, 128, 2 * CW], BF16, kind="ExternalInput").ap()
    hsD = nc.dram_tensor("hsT", [NCH, 128, HSW], BF16, kind="ExternalInput").ap()
    wAD = nc.dram_tensor("wA", [128, WA_COLS], BF16, kind="ExternalInput").ap()
    wBD = nc.dram_tensor("wB", [128, WB_COLS], BF16, kind="ExternalInput").ap()
    biasp = nc.dram_tensor("biasp", [128, 6], F32, kind="ExternalInput").ap()
    outD = nc.dram_tensor("outT", [NCH, 128, 2 * CW], F32,
                          kind="ExternalOutput").ap()

    with tile.TileContext(nc) as tc, ExitStack() as ctx:
        const_pool = ctx.enter_context(tc.tile_pool(name="const", bufs=1))
        x_pool = ctx.enter_context(tc.tile_pool(name="x", bufs=3))
        h_pool = ctx.enter_context(tc.tile_pool(name="h", bufs=3))
        hb_pool = ctx.enter_context(tc.tile_pool(name="hb", bufs=2))
        hs_pool = ctx.enter_context(tc.tile_pool(name="hs", bufs=3))
        xr_pool = ctx.enter_context(tc.tile_pool(name="xr", bufs=2))
        r_pool = ctx.enter_context(tc.tile_pool(name="r", bufs=2))
        s_pool = ctx.enter_context(tc.tile_pool(name="s", bufs=2))
        z_pool = ctx.enter_context(tc.tile_pool(name="z", bufs=2))
        n_pool = ctx.enter_context(tc.tile_pool(name="n", bufs=2))
        d_pool = ctx.enter_context(tc.tile_pool(name="d", bufs=2))
        o_pool = ctx.enter_context(tc.tile_pool(name="o", bufs=2))
        ps_pool = ctx.enter_context(tc.tile_pool(name="ps", bufs=4, space="PSUM"))

        # --- dependency-free warm-up (HAM clock gate + ACT table load) ---
        junk = const_pool.tile([128, 512], BF16, tag="junk", name="junk")
        nc.vector.memset(junk[:, :], 0)
        # enough filler to keep the PE busy-window continuous through the
        # first chunk's DMA wait, so the real stream starts at K=8/8
        warm_ps = ps_pool.tile([128, 2 * CW], F32, tag="ps", name="warm_ps")
        for i in range(13):
            nc.tensor.matmul(warm_ps[:, 0:CW], junk[:, 0:128], junk[:, :],
                             start=True, stop=True)
        warm_act = const_pool.tile([128, 4], F32, tag="wact", name="warm_act")
        nc.scalar.activation(warm_act[:, 0:1], junk[:, 0:1], SIG)
        nc.scalar.activation(warm_act[:, 1:2], junk[:, 0:1], TANH)

        # --- consts: 2 packed weight DMAs + bias, split across both
        #     HWDGE rings (sync / scalar) ---
        wA = const_pool.tile([128, WA_COLS], BF16, tag="wA", name="wA")
        nc.sync.dma_start(out=wA[:, :], in_=wAD[:, :])
        bias_t = const_pool.tile([128, 6], F32, tag="biasp", name="bias_t")
        nc.scalar.dma_start(out=bias_t[:, :], in_=biasp[:, :])
        wB = const_pool.tile([128, WB_COLS], BF16, tag="wB", name="wB")
        nc.scalar.dma_start(out=wB[:, :], in_=wBD[:, :])

        id_t = wA[:, 0:128]
        woff = {}
        for i, w in enumerate(WA_ORDER):
            woff[w] = (wA, 128 + i * 512)
        for i, w in enumerate(WB_ORDER):
            woff[w] = (wB, i * 512)

        def wsl(w, k, f):      # stationary [128,128]: contract chunk k, out chunk f
            t, off = woff[w]
            base = off + k * 256 + f * 128
            return t[:, base:base + 128]

        # Steps: 7 full-width chunks + the last DRAM chunk as two
        # half-width steps, shrinking the end-of-kernel serial chain.
        # Tiles keep full-chunk geometry; half steps pack/use a cw-wide
        # prefix of each tile.
        steps = [(c, 0, CW) for c in range(NCH - 1)] + \
                [(NCH - 1, 0, CW // 2), (NCH - 1, CW // 2, CW // 2)]
        NS = len(steps)

        st = {}   # per-step state carried from head(s) to tail(s)
        nts = {}

        def tail_n(s, warm=0, split_x=False):
            xt, wz, tz, sc, cw = st.pop(s)

            def xk(k):
                return xt[:, k * cw:(k + 1) * cw]

            # -- n = tanh(Win@x + Whn@s + b_n) -> f32 SBUF --
            nt = n_pool.tile([128, 2 * CW], F32, tag="n", name=f"n_{s}")
            pn = ps_pool.tile([128, 2 * CW], F32, tag="ps", name=f"pn_{s}")
            for i in range(warm):
                # filler matmuls: keep the PE HAM clock-gate warm while
                # the last chunk's DVE tree finishes (results overwritten)
                nc.tensor.matmul(pn[:, 0:CW], junk[:, 0:128], junk[:, :],
                                 start=True, stop=True)

            def psl(f):        # PSUM regions stay bank-aligned at any width
                return slice(f * CW, f * CW + cw)

            if split_x:
                # last step: the x-side matmuls don't need the DVE tree --
                # run them during the tree wait, leaving only the s-side
                # and tanh on the critical chain
                for f in range(2):
                    nc.tensor.matmul(pn[:, psl(f)], wsl("win", 0, f),
                                     xk(0), start=True, stop=False)
                    nc.tensor.matmul(pn[:, psl(f)], wsl("win", 1, f),
                                     xk(1), start=False, stop=False)
                for f in range(2):
                    nc.tensor.matmul(pn[:, psl(f)], wsl("whn", 0, f),
                                     sc[:, 0:cw], start=False, stop=False)
                    nc.tensor.matmul(pn[:, psl(f)], wsl("whn", 1, f),
                                     sc[:, cw:2 * cw], start=False, stop=True)
                    nc.scalar.activation(nt[:, f * cw:(f + 1) * cw],
                                         pn[:, psl(f)], TANH,
                                         bias=bias_t[:, f * 3 + 2:f * 3 + 3])
            else:
                for f in range(2):
                    nc.tensor.matmul(pn[:, psl(f)], wsl("win", 0, f),
                                     xk(0), start=True, stop=False)
                    nc.tensor.matmul(pn[:, psl(f)], wsl("win", 1, f),
                                     xk(1), start=False, stop=False)
                    nc.tensor.matmul(pn[:, psl(f)], wsl("whn", 0, f),
                                     sc[:, 0:cw], start=False, stop=False)
                    nc.tensor.matmul(pn[:, psl(f)], wsl("whn", 1, f),
                                     sc[:, cw:2 * cw], start=False, stop=True)
                for f in range(2):
                    nc.scalar.activation(nt[:, f * cw:(f + 1) * cw],
                                         pn[:, f * CW:f * CW + cw], TANH,
                                         bias=bias_t[:, f * 3 + 2:f * 3 + 3])
            nts[s] = (nt, wz, tz, cw)

        def out_slice(ci, b0, cw, f):
            base = f * CW + b0
            return outD[ci, :, base:base + cw]

        def tail_combine(s, split=False):
            nt, wz, tz, cw = nts.pop(s)
            ci, b0, _ = steps[s]
            # -- out = n*(1-z) + z*h, fp32 on DVE; (1-z) and z*h were
            #    precomputed in head(s) so the tail is 2 ops --
            dt_ = d_pool.tile([128, 2 * CW], F32, tag="d", name=f"d_{s}")
            ot = o_pool.tile([128, 2 * CW], F32, tag="o", name=f"o_{s}")
            if not split:
                nc.vector.tensor_mul(dt_[:, 0:2 * cw], nt[:, 0:2 * cw],
                                     wz[:, 0:2 * cw])
                nc.vector.tensor_add(ot[:, 0:2 * cw], dt_[:, 0:2 * cw],
                                     tz[:, 0:2 * cw])
                if cw == CW:
                    nc.sync.dma_start(out=outD[ci], in_=ot[:, :])
                else:
                    for f in range(2):
                        nc.sync.dma_start(
                            out=out_slice(ci, b0, cw, f),
                            in_=ot[:, f * cw:(f + 1) * cw])
            else:
                # last step: pipeline per f-half behind the tanh drains
                for f in range(2):
                    sl = slice(f * cw, (f + 1) * cw)
                    nc.vector.tensor_mul(dt_[:, sl], nt[:, sl], wz[:, sl])
                    nc.vector.tensor_add(ot[:, sl], dt_[:, sl], tz[:, sl])
                    nc.sync.dma_start(out=out_slice(ci, b0, cw, f),
                                      in_=ot[:, sl])

        def tail(s):
            tail_n(s)
            tail_combine(s)

        dma_tiles = {}

        def emit_dmas(s):
            ci, b0, cw = steps[s]
            full = cw == CW
            xt = x_pool.tile([128, 2 * CW], BF16, tag="x", name=f"x_{s}")
            htb = hb_pool.tile([128, 2 * CW], BF16, tag="hb", name=f"hb_{s}")
            ht = h_pool.tile([128, 2 * CW], F32, tag="h", name=f"h_{s}")
            hsc = hs_pool.tile([128, HSW], BF16, tag="hs", name=f"hs_{s}")
            if full:
                nc.sync.dma_start(out=xt[:, :], in_=xD[ci])
                nc.sync.dma_start(out=htb[:, :], in_=hbD[ci])
                # hs split by neighbor half, each DMA covering both
                # k-chunks, so r-units can start on the first transfer
                nparts = 4 if s == 0 else 2
                HQ = N_NEIGH * CW // nparts
                for piece in range(nparts):
                    nc.sync.dma_start(
                        out=hsc[:, :].rearrange("p (k x) -> p k x", k=2)
                            [:, :, piece * HQ:(piece + 1) * HQ],
                        in_=hsD[ci].rearrange("p (k x) -> p k x", k=2)
                            [:, :, piece * HQ:(piece + 1) * HQ])
                nc.sync.dma_start(out=ht[:, :], in_=hD[ci])
            else:
                bsl = slice(b0, b0 + cw)
                nc.sync.dma_start(
                    out=xt[:, 0:2 * cw].rearrange("p (k b) -> p k b", b=cw),
                    in_=xD[ci].rearrange("p (k b) -> p k b", b=CW)[:, :, bsl])
                nc.sync.dma_start(
                    out=htb[:, 0:2 * cw].rearrange("p (k b) -> p k b", b=cw),
                    in_=hbD[ci].rearrange("p (k b) -> p k b", b=CW)[:, :, bsl])
                nc.sync.dma_start(
                    out=hsc[:, 0:16 * cw].rearrange("p (s b) -> p s b", b=cw),
                    in_=hsD[ci].rearrange("p (s b) -> p s b", b=CW)[:, :, bsl])
                nc.sync.dma_start(
                    out=ht[:, 0:2 * cw].rearrange("p (k b) -> p k b", b=cw),
                    in_=hD[ci].rearrange("p (k b) -> p k b", b=CW)[:, :, bsl])
            dma_tiles[s] = (xt, htb, ht, hsc)

        def head(s):
            ci, b0, cw = steps[s]
            if s + 1 < NS:
                emit_dmas(s + 1)        # prefetch next step's inputs
            xt, htb, ht, hsc = dma_tiles.pop(s)

            def xk(k):
                return xt[:, k * cw:(k + 1) * cw]

            def hs_sl(k, n):   # hs layout (k, n, b): [128, cw] matmul operand
                base = (k * N_NEIGH + n) * cw
                return hsc[:, base:base + cw]

            # -- xr = Wir@x + b_r  -> bf16 SBUF --
            xr = xr_pool.tile([128, 2 * CW], BF16, tag="xr", name=f"xr_{s}")
            pxr = ps_pool.tile([128, 2 * CW], F32, tag="ps", name=f"pxr_{s}")
            for f in range(2):
                for k in range(2):
                    nc.tensor.matmul(pxr[:, f * CW:f * CW + cw],
                                     wsl("wir", k, f), xk(k),
                                     start=(k == 0), stop=(k == 1))
            for f in range(2):
                nc.scalar.activation(xr[:, f * cw:(f + 1) * cw],
                                     pxr[:, f * CW:f * CW + cw], IDENT,
                                     bias=bias_t[:, f * 3:f * 3 + 1])

            # -- z = sigmoid(Wiz@x + Whz@h + b_z) -> f32 SBUF --
            zt = z_pool.tile([128, 2 * CW], F32, tag="z", name=f"z_{s}")
            pz = ps_pool.tile([128, 2 * CW], F32, tag="ps", name=f"pz_{s}")
            for f in range(2):
                psl = slice(f * CW, f * CW + cw)
                nc.tensor.matmul(pz[:, psl], wsl("wiz", 0, f),
                                 xk(0), start=True, stop=False)
                nc.tensor.matmul(pz[:, psl], wsl("wiz", 1, f),
                                 xk(1), start=False, stop=False)
                nc.tensor.matmul(pz[:, psl], wsl("whz", 0, f),
                                 htb[:, 0:cw], start=False, stop=False)
                nc.tensor.matmul(pz[:, psl], wsl("whz", 1, f),
                                 htb[:, cw:2 * cw], start=False, stop=True)
            for f in range(2):
                nc.scalar.activation(zt[:, f * cw:(f + 1) * cw],
                                     pz[:, f * CW:f * CW + cw], SIG,
                                     bias=bias_t[:, f * 3 + 1:f * 3 + 2])
            if s == 0:
                # filler matmuls bridge the first hs DMA wait so the PE
                # HAM clock-gate warms before the r-unit stream begins
                pw = ps_pool.tile([128, 2 * CW], F32, tag="ps",
                                  name="pwarm0")
                for i in range(7):
                    nc.tensor.matmul(pw[:, 0:CW], junk[:, 0:128], junk[:, :],
                                     start=True, stop=True)
            # precompute combine terms: wz = 1-z, tz = z*h (hides in head)
            wz = z_pool.tile([128, 2 * CW], F32, tag="wz", name=f"wz_{s}")
            nc.vector.tensor_scalar(wz[:, 0:2 * cw], zt[:, 0:2 * cw],
                                    -1.0, 1.0,
                                    mybir.AluOpType.mult, mybir.AluOpType.add)
            tz = z_pool.tile([128, 2 * CW], F32, tag="tz", name=f"tz_{s}")
            nc.vector.tensor_mul(tz[:, 0:2 * cw], zt[:, 0:2 * cw],
                                 ht[:, 0:2 * cw])

            # -- r units: (neighbor pair j, out chunk f) [128, 2*cw] PSUM --
            rc = r_pool.tile([128, HSW], BF16, tag="r", name=f"r_{s}")
            sc = s_pool.tile([128, 2 * CW], BF16, tag="s", name=f"s_{s}")

            def r_unit(j, f):
                pr = ps_pool.tile([128, 2 * CW], F32, tag="ps",
                                  name=f"pr{f}{j}_{s}")
                for k in range(2):
                    nc.tensor.matmul(pr[:, 0:cw], wsl("whr", k, f),
                                     hs_sl(k, 2 * j), start=(k == 0),
                                     stop=False)
                    nc.tensor.matmul(pr[:, CW:CW + cw], wsl("whr", k, f),
                                     hs_sl(k, 2 * j + 1), start=(k == 0),
                                     stop=False)
                nc.tensor.matmul(pr[:, 0:cw], id_t,
                                 xr[:, f * cw:(f + 1) * cw],
                                 start=False, stop=True)
                nc.tensor.matmul(pr[:, CW:CW + cw], id_t,
                                 xr[:, f * cw:(f + 1) * cw],
                                 start=False, stop=True)
                base = f * N_NEIGH * cw + 2 * j * cw
                nc.scalar.activation(
                    rc[:, base:base + 2 * cw]
                        .rearrange("p (g b) -> p g b", g=2),
                    pr[:, :].rearrange("p (g b) -> p g b", g=2)[:, :, 0:cw],
                    SIG)

            def f_half(f):
                fb = f * N_NEIGH * cw
                q = 2 * cw                    # 2 neighbors
                r_unit(0, f)
                r_unit(1, f)
                # products for neighbors 0-3 + pair-tree, in place in rc
                nc.vector.tensor_mul(rc[:, fb:fb + 2 * q],
                                     rc[:, fb:fb + 2 * q],
                                     hsc[:, fb:fb + 2 * q])
                with nc.allow_low_precision(reason="bf16 neighbor sums"):
                    nc.vector.tensor_add(rc[:, fb:fb + q], rc[:, fb:fb + q],
                                         rc[:, fb + q:fb + 2 * q])
                r_unit(2, f)
                r_unit(3, f)
                nc.vector.tensor_mul(rc[:, fb + 2 * q:fb + 4 * q],
                                     rc[:, fb + 2 * q:fb + 4 * q],
                                     hsc[:, fb + 2 * q:fb + 4 * q])
                with nc.allow_low_precision(reason="bf16 neighbor sums"):
                    nc.vector.tensor_add(rc[:, fb + 2 * q:fb + 3 * q],
                                         rc[:, fb + 2 * q:fb + 3 * q],
                                         rc[:, fb + 3 * q:fb + 4 * q])
                    nc.vector.tensor_add(rc[:, fb:fb + q], rc[:, fb:fb + q],
                                         rc[:, fb + 2 * q:fb + 3 * q])
                    nc.vector.tensor_add(sc[:, f * cw:(f + 1) * cw],
                                         rc[:, fb:fb + cw],
                                         rc[:, fb + cw:fb + 2 * cw])

            def f_half_fast(f):
                # last step: running accumulation into sc as each r-unit
                # drains, so only ~1us of DVE work follows the final
                # sigmoid (vs the bulk tree)
                fb = f * N_NEIGH * cw
                q = 2 * cw
                r_unit(0, f)
                r_unit(1, f)
                nc.vector.tensor_mul(rc[:, fb:fb + 2 * q],
                                     rc[:, fb:fb + 2 * q],
                                     hsc[:, fb:fb + 2 * q])
                with nc.allow_low_precision(reason="bf16 neighbor sums"):
                    nc.vector.tensor_add(rc[:, fb:fb + q], rc[:, fb:fb + q],
                                         rc[:, fb + q:fb + 2 * q])
                    nc.vector.tensor_add(sc[:, f * cw:(f + 1) * cw],
                                         rc[:, fb:fb + cw],
                                         rc[:, fb + cw:fb + 2 * cw])
                for j in (2, 3):
                    r_unit(j, f)
                    ub = fb + j * q
                    nc.vector.tensor_mul(rc[:, ub:ub + q], rc[:, ub:ub + q],
                                         hsc[:, ub:ub + q])
                    with nc.allow_low_precision(reason="bf16 neighbor sums"):
                        nc.vector.tensor_add(rc[:, ub:ub + cw],
                                             rc[:, ub:ub + cw],
                                             rc[:, ub + cw:ub + q])
                        nc.vector.tensor_add(sc[:, f * cw:(f + 1) * cw],
                                             sc[:, f * cw:(f + 1) * cw],
                                             rc[:, ub:ub + cw])

            fh = f_half_fast if s == NS - 1 else f_half
            fh(0)
            st[s] = (xt, wz, tz, sc, cw)
            # overlap previous step's tail with f=1; for the last step
            # only the n-gate goes between halves (its combine would
            # delay the final DVE tree)
            if 1 <= s < NS - 1:
                tail(s - 1)
            elif s == NS - 1:
                tail_n(s - 1)
            fh(1)

        emit_dmas(0)
        for s in range(NS):
            head(s)
        tail_combine(NS - 2)
        tail_n(NS - 1, warm=5, split_x=True)
        tail_combine(NS - 1, split=True)

    nc.compile()
    return nc


def _prep_inputs(x, h_sum, hs, Wir, bir, Whr, bhr, Wiz, biz, Whz, bhz,
                 Win, bin_, Whn, bhn):
    """Shard + pack to per-core, per-chunk SBUF-image layouts."""
    f32 = np.float32

    wt = {}
    for name, W in (("wir", Wir), ("whr", Whr), ("wiz", Wiz), ("whz", Whz),
                    ("win", Win), ("whn", Whn)):
        wt[name] = np.asarray(W, f32).T.astype(BF_NP)   # [256 in, 256 out]
    wApack = np.empty((128, WA_COLS), BF_NP)
    wApack[:, 0:128] = np.eye(128, dtype=f32).astype(BF_NP)
    for i, w in enumerate(WA_ORDER):
        for k in range(2):
            wApack[:, 128 + i * 512 + k * 256: 128 + i * 512 + (k + 1) * 256] \
                = wt[w][k * 128:(k + 1) * 128, :]
    wBpack = np.empty((128, WB_COLS), BF_NP)
    for i, w in enumerate(WB_ORDER):
        for k in range(2):
            wBpack[:, i * 512 + k * 256: i * 512 + (k + 1) * 256] \
                = wt[w][k * 128:(k + 1) * 128, :]

    b_r = np.asarray(bir, f32) + np.asarray(bhr, f32)
    b_z = np.asarray(biz, f32) + np.asarray(bhz, f32)
    b_n = np.asarray(bin_, f32) + np.asarray(bhn, f32)
    biasp = np.empty((128, 6), f32)
    for f in range(2):
        biasp[:, f * 3 + 0] = b_r[f * 128:(f + 1) * 128]
        biasp[:, f * 3 + 1] = b_z[f * 128:(f + 1) * 128]
        biasp[:, f * 3 + 2] = b_n[f * 128:(f + 1) * 128]

    # x: [B, 256] -> per core [NCH, 128, (k, b)] bf16
    xbf = np.asarray(x, f32).astype(BF_NP)
    x5 = xbf.reshape(M, NCH, CW, 2, 128)            # [core, c, b, k, p]
    x_pack = np.ascontiguousarray(x5.transpose(0, 1, 4, 3, 2)) \
        .reshape(M, NCH, 128, 2 * CW)
    hf = np.asarray(h_sum, f32)
    h5 = hf.reshape(M, NCH, CW, 2, 128)
    h_pack = np.ascontiguousarray(h5.transpose(0, 1, 4, 3, 2)) \
        .reshape(M, NCH, 128, 2 * CW)
    hb_pack = np.ascontiguousarray(h_pack.astype(BF_NP))
    # hs: [8, B, 256] -> per core [NCH, 128, (k, n, b)] bf16
    hsbf = np.asarray(hs, f32).astype(BF_NP)
    hs6 = hsbf.reshape(N_NEIGH, M, NCH, CW, 2, 128)  # [n, core, c, b, k, p]
    hs_pack = np.ascontiguousarray(hs6.transpose(1, 2, 5, 4, 0, 3)) \
        .reshape(M, NCH, 128, HSW)

    in_maps = []
    for core in range(M):
        m = {
            "xT": x_pack[core],
            "hT": h_pack[core],
            "hbT": hb_pack[core],
            "hsT": hs_pack[core],
            "wA": wApack,
            "wB": wBpack,
            "biasp": biasp,
        }
        in_maps.append(m)
    return in_maps


def _run(inputs, trace=False, **trace_kwargs):
    global _cached
    if _cached is None:
        _cached = _build()
    nc = _cached
    in_maps = _prep_inputs(**inputs)
    res = run_bass_kernel_spmd(nc, in_maps, list(range(M)), trace=trace,
                               **trace_kwargs)
    out = np.empty((B, H), np.float32)
    for core in range(M):
        o = res.results[core]["outT"]          # [NCH, 128, (f, b)] f32
        o = o.reshape(NCH, 128, 2, CW).transpose(0, 3, 2, 1)  # [c, b, f, p]
        out[core * BL:(core + 1) * BL, :] = o.reshape(BL, H)
    return out, res


def kernel(**inputs):
    return _run(inputs)[0]


# revision 41
# speedup vs baseline: 1.0030x; 1.0030x over previous
"""GRU-style GNN message-passing kernel for Trainium2 (8 NeuronCores, SPMD).

Reference computation (per node b, features 256, 8 neighbors):
    xr = x @ Wir.T + bir
    hr_n = hs_n @ Whr.T + bhr
    r_n = sigmoid(xr + hr_n)
    z = sigmoid(x @ Wiz.T + biz + h_sum @ Whz.T + bhz)
    s = sum_n r_n * hs_n
    n = tanh(x @ Win.T + bin + s @ Whn.T + bhn)
    out = (1 - z) * n + z * h_sum

Strategy: data-parallel over the node dim B=32768 across 8 cores (4096
rows each), batch-chunked 8x512 per core, feature-major on chip
(256 feats = 2 partition chunks of 128, batch in the free dim).

Performance structure:
  - All HBM tensors are host-packed into the exact SBUF image per chunk;
    the 13 weight/const tensors ride in 2 packed DMAs so the first
    chunk's data isn't stuck behind a serialized descriptor queue
    (HWDGE issue is ~650ns per dma_start). hs is split into two per-f
    DMAs so the first r-matmuls start before the full 2MB lands.
  - Software-pipelined emission: chunk c's n-gate + combine + store are
    emitted *between* the f=0 and f=1 r-unit groups of chunk c+1, so
    the PE stream never waits on the DVE product/sum tree.
  - Product+tree on DVE is split (neighbors 0-3 / 4-7) so it pipelines
    against the second half of the r-matmuls; the final combine runs on
    the otherwise-idle GPSIMD except for the last chunk (short tail).
  - PSUM: one pool of 4 x [128,1024] tiles (8 banks); xr, z, 8 r-units
    and the (delayed) n-gate rotate through it.
  - Dependency-free warm-up matmuls + activations at t=0 keep the PE
    HAM clock-gate and ACT tables warm through the initial DMA wait.
"""

import sys
import numpy as np
from contextlib import ExitStack

sys.path.insert(0, "/opt/trn_rl_repo")

import ml_dtypes
import concourse.bacc as bacc
import concourse.tile as tile
from concourse import mybir
from concourse.bass_utils import run_bass_kernel_spmd

F32 = mybir.dt.float32
BF16 = mybir.dt.bfloat16
BF_NP = ml_dtypes.bfloat16

N_NEIGH, B, IN, H = 8, 32768, 256, 256
M = 8                    # cores
BL = B // M              # rows per core (4096)
NCH = 8                  # batch chunks per core
CW = BL // NCH           # chunk width (512)
HSW = 2 * N_NEIGH * CW   # hs tile width (8192)

SIG = mybir.ActivationFunctionType.Sigmoid
TANH = mybir.ActivationFunctionType.Tanh
IDENT = mybir.ActivationFunctionType.Identity

# packed const layout: [id | wir0 wir1 wiz0 wiz1 whz0 whz1] / [whr0 whr1 win0 win1 whn0 whn1]
WA_ORDER = ("wir", "wiz", "whz")
WB_ORDER = ("whr", "win", "whn")
WA_COLS = 128 + 6 * 256
WB_COLS = 6 * 256

_cached = None  # compiled program, reused across kernel() calls


def _build():
    nc = bacc.Bacc("TRN2", target_bir_lowering=False, debug=False, num_devices=M)

    xD = nc.dram_tensor("xT", [NCH, 128, 2 * CW], BF16, kind="ExternalInput").ap()
    hD = nc.dram_tensor("hT", [NCH, 128, 2 * CW], F32, kind="ExternalInput").ap()
    hbD = nc.dram_tensor("hbT", [NCH, 128, 2 * CW], BF16, kind="ExternalInput").ap()
    hsD = nc.dram_tensor("hsT", [NCH, 128, HSW], BF16, kind="ExternalInput").ap()
    wAD = nc.dram_tensor("wA", [128, WA_COLS], BF16, kind="ExternalInput").ap()
    wBD = nc.dram_tensor("wB", [128, WB_COLS], BF16, kind="ExternalInput").ap()
    biasp = nc.dram_tensor("biasp", [128, 6], F32, kind="ExternalInput").ap()
    outD = nc.dram_tensor("outT", [NCH, 128, 2 * CW], F32,
                          kind="ExternalOutput").ap()

    with tile.TileContext(nc) as tc, ExitStack() as ctx:
        const_pool = ctx.enter_context(tc.tile_pool(name="const", bufs=1))
        x_pool = ctx.enter_context(tc.tile_pool(name="x", bufs=3))
        h_pool = ctx.enter_context(tc.tile_pool(name="h", bufs=3))
        hb_pool = ctx.enter_context(tc.tile_pool(name="hb", bufs=2))
        hs_pool = ctx.enter_context(tc.tile_pool(name="hs", bufs=3))
        xr_pool = ctx.enter_context(tc.tile_pool(name="xr", bufs=2))
        r_pool = ctx.enter_context(tc.tile_pool(name="r", bufs=2))
        s_pool = ctx.enter_context(tc.tile_pool(name="s", bufs=2))
        z_pool = ctx.enter_context(tc.tile_pool(name="z", bufs=2))
        n_pool = ctx.enter_context(tc.tile_pool(name="n", bufs=2))
        d_pool = ctx.enter_context(tc.tile_pool(name="d", bufs=2))
        o_pool = ctx.enter_context(tc.tile_pool(name="o", bufs=2))
        ps_pool = ctx.enter_context(tc.tile_pool(name="ps", bufs=4, space="PSUM"))

        # --- dependency-free warm-up (HAM clock gate + ACT table load) ---
        junk = const_pool.tile([128, 512], BF16, tag="junk", name="junk")
        nc.vector.memset(junk[:, :], 0)
        # enough filler to keep the PE busy-window continuous through the
        # first chunk's DMA wait, so the real stream starts at K=8/8
        warm_ps = ps_pool.tile([128, 2 * CW], F32, tag="ps", name="warm_ps")
        for i in range(13):
            nc.tensor.matmul(warm_ps[:, 0:CW], junk[:, 0:128], junk[:, :],
                             start=True, stop=True)
        warm_act = const_pool.tile([128, 4], F32, tag="wact", name="warm_act")
        nc.scalar.activation(warm_act[:, 0:1], junk[:, 0:1], SIG)
        nc.scalar.activation(warm_act[:, 1:2], junk[:, 0:1], TANH)

        # --- consts: 2 packed weight DMAs + bias, split across both
        #     HWDGE rings (sync / scalar) ---
        wA = const_pool.tile([128, WA_COLS], BF16, tag="wA", name="wA")
        nc.sync.dma_start(out=wA[:, :], in_=wAD[:, :])
        bias_t = const_pool.tile([128, 6], F32, tag="biasp", name="bias_t")
        nc.scalar.dma_start(out=bias_t[:, :], in_=biasp[:, :])
        wB = const_pool.tile([128, WB_COLS], BF16, tag="wB", name="wB")
        nc.scalar.dma_start(out=wB[:, :], in_=wBD[:, :])

        id_t = wA[:, 0:128]
        woff = {}
        for i, w in enumerate(WA_ORDER):
            woff[w] = (wA, 128 + i * 512)
        for i, w in enumerate(WB_ORDER):
            woff[w] = (wB, i * 512)

        def wsl(w, k, f):      # stationary [128,128]: contract chunk k, out chunk f
            t, off = woff[w]
            base = off + k * 256 + f * 128
            return t[:, base:base + 128]

        # Steps: 7 full-width chunks + the last DRAM chunk as two
        # half-width steps, shrinking the end-of-kernel serial chain.
        # Tiles keep full-chunk geometry; half steps pack/use a cw-wide
        # prefix of each tile.
        steps = [(c, 0, CW) for c in range(NCH - 1)] + \
                [(NCH - 1, 0, CW // 2), (NCH - 1, CW // 2, CW // 2)]
        NS = len(steps)

        st = {}   # per-step state carried from head(s) to tail(s)
        nts = {}

        def tail_n(s, warm=0):
            xt, wz, tz, sc, cw = st.pop(s)

            def xk(k):
                return xt[:, k * cw:(k + 1) * cw]

            # -- n = tanh(Win@x + Whn@s + b_n) -> f32 SBUF --
            nt = n_pool.tile([128, 2 * CW], F32, tag="n", name=f"n_{s}")
            pn = ps_pool.tile([128, 2 * CW], F32, tag="ps", name=f"pn_{s}")
            for i in range(warm):
                # filler matmuls: keep the PE HAM clock-gate warm while
                # the last chunk's DVE tree finishes (results overwritten)
                nc.tensor.matmul(pn[:, 0:CW], junk[:, 0:128], junk[:, :],
                                 start=True, stop=True)
            for f in range(2):
                # PSUM regions stay bank-aligned (f*CW) at any width
                psl = slice(f * CW, f * CW + cw)
                nc.tensor.matmul(pn[:, psl], wsl("win", 0, f),
                                 xk(0), start=True, stop=False)
                nc.tensor.matmul(pn[:, psl], wsl("win", 1, f),
                                 xk(1), start=False, stop=False)
                nc.tensor.matmul(pn[:, psl], wsl("whn", 0, f),
                                 sc[:, 0:cw], start=False, stop=False)
                nc.tensor.matmul(pn[:, psl], wsl("whn", 1, f),
                                 sc[:, cw:2 * cw], start=False, stop=True)
            for f in range(2):
                nc.scalar.activation(nt[:, f * cw:(f + 1) * cw],
                                     pn[:, f * CW:f * CW + cw], TANH,
                                     bias=bias_t[:, f * 3 + 2:f * 3 + 3])
            nts[s] = (nt, wz, tz, cw)

        def out_slice(ci, b0, cw, f):
            base = f * CW + b0
            return outD[ci, :, base:base + cw]

        def tail_combine(s, split=False):
            nt, wz, tz, cw = nts.pop(s)
            ci, b0, _ = steps[s]
            # -- out = n*(1-z) + z*h, fp32 on DVE; (1-z) and z*h were
            #    precomputed in head(s) so the tail is 2 ops --
            dt_ = d_pool.tile([128, 2 * CW], F32, tag="d", name=f"d_{s}")
            ot = o_pool.tile([128, 2 * CW], F32, tag="o", name=f"o_{s}")
            if not split:
                nc.vector.tensor_mul(dt_[:, 0:2 * cw], nt[:, 0:2 * cw],
                                     wz[:, 0:2 * cw])
                nc.vector.tensor_add(ot[:, 0:2 * cw], dt_[:, 0:2 * cw],
                                     tz[:, 0:2 * cw])
                if cw == CW:
                    nc.sync.dma_start(out=outD[ci], in_=ot[:, :])
                else:
                    for f in range(2):
                        nc.sync.dma_start(
                            out=out_slice(ci, b0, cw, f),
                            in_=ot[:, f * cw:(f + 1) * cw])
            else:
                # last step: pipeline per f-half behind the tanh drains
                for f in range(2):
                    sl = slice(f * cw, (f + 1) * cw)
                    nc.vector.tensor_mul(dt_[:, sl], nt[:, sl], wz[:, sl])
                    nc.vector.tensor_add(ot[:, sl], dt_[:, sl], tz[:, sl])
                    nc.sync.dma_start(out=out_slice(ci, b0, cw, f),
                                      in_=ot[:, sl])

        def tail(s):
            tail_n(s)
            tail_combine(s)

        dma_tiles = {}

        def emit_dmas(s):
            ci, b0, cw = steps[s]
            full = cw == CW
            xt = x_pool.tile([128, 2 * CW], BF16, tag="x", name=f"x_{s}")
            htb = hb_pool.tile([128, 2 * CW], BF16, tag="hb", name=f"hb_{s}")
            ht = h_pool.tile([128, 2 * CW], F32, tag="h", name=f"h_{s}")
            hsc = hs_pool.tile([128, HSW], BF16, tag="hs", name=f"hs_{s}")
            if full:
                nc.sync.dma_start(out=xt[:, :], in_=xD[ci])
                nc.sync.dma_start(out=htb[:, :], in_=hbD[ci])
                # hs split by neighbor half, each DMA covering both
                # k-chunks, so r-units can start on the first transfer
                nparts = 4 if s == 0 else 2
                HQ = N_NEIGH * CW // nparts
                for piece in range(nparts):
                    nc.sync.dma_start(
                        out=hsc[:, :].rearrange("p (k x) -> p k x", k=2)
                            [:, :, piece * HQ:(piece + 1) * HQ],
                        in_=hsD[ci].rearrange("p (k x) -> p k x", k=2)
                            [:, :, piece * HQ:(piece + 1) * HQ])
                nc.sync.dma_start(out=ht[:, :], in_=hD[ci])
            else:
                bsl = slice(b0, b0 + cw)
                nc.sync.dma_start(
                    out=xt[:, 0:2 * cw].rearrange("p (k b) -> p k b", b=cw),
                    in_=xD[ci].rearrange("p (k b) -> p k b", b=CW)[:, :, bsl])
                nc.sync.dma_start(
                    out=htb[:, 0:2 * cw].rearrange("p (k b) -> p k b", b=cw),
                    in_=hbD[ci].rearrange("p (k b) -> p k b", b=CW)[:, :, bsl])
                nc.sync.dma_start(
                    out=hsc[:, 0:16 * cw].rearrange("p (s b) -> p s b", b=cw),
                    in_=hsD[ci].rearrange("p (s b) -> p s b", b=CW)[:, :, bsl])
                nc.sync.dma_start(
                    out=ht[:, 0:2 * cw].rearrange("p (k b) -> p k b", b=cw),
                    in_=hD[ci].rearrange("p (k b) -> p k b", b=CW)[:, :, bsl])
            dma_tiles[s] = (xt, htb, ht, hsc)

        def head(s):
            ci, b0, cw = steps[s]
            if s + 1 < NS:
                emit_dmas(s + 1)        # prefetch next step's inputs
            xt, htb, ht, hsc = dma_tiles.pop(s)

            def xk(k):
                return xt[:, k * cw:(k + 1) * cw]

            def hs_sl(k, n):   # hs layout (k, n, b): [128, cw] matmul operand
                base = (k * N_NEIGH + n) * cw
                return hsc[:, base:base + cw]

            # -- xr = Wir@x + b_r  -> bf16 SBUF --
            xr = xr_pool.tile([128, 2 * CW], BF16, tag="xr", name=f"xr_{s}")
            pxr = ps_pool.tile([128, 2 * CW], F32, tag="ps", name=f"pxr_{s}")
            for f in range(2):
                for k in range(2):
                    nc.tensor.matmul(pxr[:, f * CW:f * CW + cw],
                                     wsl("wir", k, f), xk(k),
                                     start=(k == 0), stop=(k == 1))
            for f in range(2):
                nc.scalar.activation(xr[:, f * cw:(f + 1) * cw],
                                     pxr[:, f * CW:f * CW + cw], IDENT,
                                     bias=bias_t[:, f * 3:f * 3 + 1])

            # -- z = sigmoid(Wiz@x + Whz@h + b_z) -> f32 SBUF --
            zt = z_pool.tile([128, 2 * CW], F32, tag="z", name=f"z_{s}")
            pz = ps_pool.tile([128, 2 * CW], F32, tag="ps", name=f"pz_{s}")
            for f in range(2):
                psl = slice(f * CW, f * CW + cw)
                nc.tensor.matmul(pz[:, psl], wsl("wiz", 0, f),
                                 xk(0), start=True, stop=False)
                nc.tensor.matmul(pz[:, psl], wsl("wiz", 1, f),
                                 xk(1), start=False, stop=False)
                nc.tensor.matmul(pz[:, psl], wsl("whz", 0, f),
                                 htb[:, 0:cw], start=False, stop=False)
                nc.tensor.matmul(pz[:, psl], wsl("whz", 1, f),
                                 htb[:, cw:2 * cw], start=False, stop=True)
            for f in range(2):
                nc.scalar.activation(zt[:, f * cw:(f + 1) * cw],
                                     pz[:, f * CW:f * CW + cw], SIG,
                                     bias=bias_t[:, f * 3 + 1:f * 3 + 2])
            if s == 0:
                # filler matmuls bridge the first hs DMA wait so the PE
                # HAM clock-gate warms before the r-unit stream begins
                pw = ps_pool.tile([128, 2 * CW], F32, tag="ps",
                                  name="pwarm0")
                for i in range(7):
                    nc.tensor.matmul(pw[:, 0:CW], junk[:, 0:128], junk[:, :],
                                     start=True, stop=True)
            # precompute combine terms: wz = 1-z, tz = z*h (hides in head)
            wz = z_pool.tile([128, 2 * CW], F32, tag="wz", name=f"wz_{s}")
            nc.vector.tensor_scalar(wz[:, 0:2 * cw], zt[:, 0:2 * cw],
                                    -1.0, 1.0,
                                    mybir.AluOpType.mult, mybir.AluOpType.add)
            tz = z_pool.tile([128, 2 * CW], F32, tag="tz", name=f"tz_{s}")
            nc.vector.tensor_mul(tz[:, 0:2 * cw], zt[:, 0:2 * cw],
                                 ht[:, 0:2 * cw])

            # -- r units: (neighbor pair j, out chunk f) [128, 2*cw] PSUM --
            rc = r_pool.tile([128, HSW], BF16, tag="r", name=f"r_{s}")
            sc = s_pool.tile([128, 2 * CW], BF16, tag="s", name=f"s_{s}")

            def r_unit(j, f):
                pr = ps_pool.tile([128, 2 * CW], F32, tag="ps",
                                  name=f"pr{f}{j}_{s}")
                for k in range(2):
                    nc.tensor.matmul(pr[:, 0:cw], wsl("whr", k, f),
                                     hs_sl(k, 2 * j), start=(k == 0),
                                     stop=False)
                    nc.tensor.matmul(pr[:, CW:CW + cw], wsl("whr", k, f),
                                     hs_sl(k, 2 * j + 1), start=(k == 0),
                                     stop=False)
                nc.tensor.matmul(pr[:, 0:cw], id_t,
                                 xr[:, f * cw:(f + 1) * cw],
                                 start=False, stop=True)
                nc.tensor.matmul(pr[:, CW:CW + cw], id_t,
                                 xr[:, f * cw:(f + 1) * cw],
                                 start=False, stop=True)
                base = f * N_NEIGH * cw + 2 * j * cw
                nc.scalar.activation(
                    rc[:, base:base + 2 * cw]
                        .rearrange("p (g b) -> p g b", g=2),
                    pr[:, :].rearrange("p (g b) -> p g b", g=2)[:, :, 0:cw],
                    SIG)

            def f_half(f):
                fb = f * N_NEIGH * cw
                q = 2 * cw                    # 2 neighbors
                r_unit(0, f)
                r_unit(1, f)
                # products for neighbors 0-3 + pair-tree, in place in rc
                nc.vector.tensor_mul(rc[:, fb:fb + 2 * q],
                                     rc[:, fb:fb + 2 * q],
                                     hsc[:, fb:fb + 2 * q])
                with nc.allow_low_precision(reason="bf16 neighbor sums"):
                    nc.vector.tensor_add(rc[:, fb:fb + q], rc[:, fb:fb + q],
                                         rc[:, fb + q:fb + 2 * q])
                r_unit(2, f)
                r_unit(3, f)
                nc.vector.tensor_mul(rc[:, fb + 2 * q:fb + 4 * q],
                                     rc[:, fb + 2 * q:fb + 4 * q],
                                     hsc[:, fb + 2 * q:fb + 4 * q])
                with nc.allow_low_precision(reason="bf16 neighbor sums"):
                    nc.vector.tensor_add(rc[:, fb + 2 * q:fb + 3 * q],
                                         rc[:, fb + 2 * q:fb + 3 * q],
                                         rc[:, fb + 3 * q:fb + 4 * q])
                    nc.vector.tensor_add(rc[:, fb:fb + q], rc[:, fb:fb + q],
                                         rc[:, fb + 2 * q:fb + 3 * q])
                    nc.vector.tensor_add(sc[:, f * cw:(f + 1) * cw],
                                         rc[:, fb:fb + cw],
                                         rc[:, fb + cw:fb + 2 * cw])

            def f_half_fast(f):
                # last step: running accumulation into sc as each r-unit
                # drains, so only ~1us of DVE work follows the final
                # sigmoid (vs the bulk tree)
                fb = f * N_NEIGH * cw
                q = 2 * cw
                r_unit(0, f)
                r_unit(1, f)
                nc.vector.tensor_mul(rc[:, fb:fb + 2 * q],
                                     rc[:, fb:fb + 2 * q],
                                     hsc[:, fb:fb + 2 * q])
                with nc.allow_low_precision(reason="bf16 neighbor sums"):
                    nc.vector.tensor_add(rc[:, fb:fb + q], rc[:, fb:fb + q],
                                         rc[:, fb + q:fb + 2 * q])
                    nc.vector.tensor_add(sc[:, f * cw:(f + 1) * cw],
                                         rc[:, fb:fb + cw],
                                         rc[:, fb + cw:fb + 2 * cw])
                for j in (2, 3):
                    r_unit(j, f)
                    ub = fb + j * q
                    nc.vector.tensor_mul(rc[:, ub:ub + q], rc[:, ub:ub + q],
                                         hsc[:, ub:ub + q])
                    with nc.allow_low_precision(reason="bf16 neighbor sums"):
                        nc.vector.tensor_add(rc[:, ub:ub + cw],
                                             rc[:, ub:ub + cw],
                                             rc[:, ub + cw:ub + q])
                        nc.vector.tensor_add(sc[:, f * cw:(f + 1) * cw],
                                             sc[:, f * cw:(f + 1) * cw],
                                             rc[:, ub:ub + cw])

            fh = f_half_fast if s == NS - 1 else f_half
            fh(0)
            st[s] = (xt, wz, tz, sc, cw)
            # overlap previous step's tail with f=1; for the last step
            # only the n-gate goes between halves (its combine would
            # delay the final DVE tree)
            if 1 <= s < NS - 1:
                tail(s - 1)
            elif s == NS - 1:
                tail_n(s - 1)
            fh(1)

        emit_dmas(0)
        for s in range(NS):
            head(s)
        tail_combine(NS - 2)
        tail_n(NS - 1, warm=5)
        tail_combine(NS - 1, split=True)

    nc.compile()
    return nc


def _prep_inputs(x, h_sum, hs, Wir, bir, Whr, bhr, Wiz, biz, Whz, bhz,
                 Win, bin_, Whn, bhn):
    """Shard + pack to per-core, per-chunk SBUF-image layouts."""
    f32 = np.float32

    wt = {}
    for name, W in (("wir", Wir), ("whr", Whr), ("wiz", Wiz), ("whz", Whz),
                    ("win", Win), ("whn", Whn)):
        wt[name] = np.asarray(W, f32).T.astype(BF_NP)   # [256 in, 256 out]
    wApack = np.empty((128, WA_COLS), BF_NP)
    wApack[:, 0:128] = np.eye(128, dtype=f32).astype(BF_NP)
    for i, w in enumerate(WA_ORDER):
        for k in range(2):
            wApack[:, 128 + i * 512 + k * 256: 128 + i * 512 + (k + 1) * 256] \
                = wt[w][k * 128:(k + 1) * 128, :]
    wBpack = np.empty((128, WB_COLS), BF_NP)
    for i, w in enumerate(WB_ORDER):
        for k in range(2):
            wBpack[:, i * 512 + k * 256: i * 512 + (k + 1) * 256] \
                = wt[w][k * 128:(k + 1) * 128, :]

    b_r = np.asarray(bir, f32) + np.asarray(bhr, f32)
    b_z = np.asarray(biz, f32) + np.asarray(bhz, f32)
    b_n = np.asarray(bin_, f32) + np.asarray(bhn, f32)
    biasp = np.empty((128, 6), f32)
    for f in range(2):
        biasp[:, f * 3 + 0] = b_r[f * 128:(f + 1) * 128]
        biasp[:, f * 3 + 1] = b_z[f * 128:(f + 1) * 128]
        biasp[:, f * 3 + 2] = b_n[f * 128:(f + 1) * 128]

    # x: [B, 256] -> per core [NCH, 128, (k, b)] bf16
    xbf = np.asarray(x, f32).astype(BF_NP)
    x5 = xbf.reshape(M, NCH, CW, 2, 128)            # [core, c, b, k, p]
    x_pack = np.ascontiguousarray(x5.transpose(0, 1, 4, 3, 2)) \
        .reshape(M, NCH, 128, 2 * CW)
    hf = np.asarray(h_sum, f32)
    h5 = hf.reshape(M, NCH, CW, 2, 128)
    h_pack = np.ascontiguousarray(h5.transpose(0, 1, 4, 3, 2)) \
        .reshape(M, NCH, 128, 2 * CW)
    hb_pack = np.ascontiguousarray(h_pack.astype(BF_NP))
    # hs: [8, B, 256] -> per core [NCH, 128, (k, n, b)] bf16
    hsbf = np.asarray(hs, f32).astype(BF_NP)
    hs6 = hsbf.reshape(N_NEIGH, M, NCH, CW, 2, 128)  # [n, core, c, b, k, p]
    hs_pack = np.ascontiguousarray(hs6.transpose(1, 2, 5, 4, 0, 3)) \
        .reshape(M, NCH, 128, HSW)

    in_maps = []
    for core in range(M):
        m = {
            "xT": x_pack[core],
            "hT": h_pack[core],
            "hbT": hb_pack[core],
            "hsT": hs_pack[core],
            "wA": wApack,
            "wB": wBpack,
            "biasp": biasp,
        }
        in_maps.append(m)
    return in_maps


def _run(inputs, trace=False, **trace_kwargs):
    global _cached
    if _cached is None:
        _cached = _build()
    nc = _cached
    in_maps = _prep_inputs(**inputs)
    res = run_bass_kernel_spmd(nc, in_maps, list(range(M)), trace=trace,
                               **trace_kwargs)
    out = np.empty((B, H), np.float32)
    for core in range(M):
        o = res.results[core]["outT"]          # [NCH, 128, (f, b)] f32
        o = o.reshape(NCH, 128, 2, CW).transpose(0, 3, 2, 1)  # [c, b, f, p]
        out[core * BL:(core + 1) * BL, :] = o.reshape(BL, H)
    return out, res


def kernel(**inputs):
    return _run(inputs)[0]
